# revision 88
# baseline (speedup 1.0000x reference)
"""Trainium2 Bass kernel for BasicSelfAttention (B=2, N=2048, C=1024, H=16, Dh=64).

Sharding: 8 cores = 2 batches x 4 head-groups. Core c handles batch c//4 and
heads [4*(c%4), 4*(c%4)+4).

v3 design (on top of the v2 all-bf16 phase-interleaved kernel):
  - qkv projection in fp8-e4m3 DoubleRow mode (0.5 PE cycles/row, 2x bf16)
    with a 3-term residual split: x ~ xh+xl, W ~ wh+wl quantized at the
    SAME scale so all terms accumulate raw in one PSUM group;
    qkv = xh@wh + xh@wl + xl@wh. The dropped xl@wl term is ~2^-8 and
    incoherent, so accuracy stays at bf16 level (rel err 3.7e-3).
  - q/k weight rows are centered host-side (head-mean removed), which
    deletes the whole device-side LN mean path: var = mean(q_c^2), and
    the LN application is a single broadcast multiply.
  - W scaled x32 so the fp8 residual splits stay in e4m3 normal range;
    LN is scale-invariant (eps' = eps*32^2 folded into the Ln bias), the
    v-path scale folds into W_proj/32 host-side.
  - rstd = exp(-0.5*ln(var/64+eps')): Ln/Exp/Copy/Square share one ACT
    table (id 6), so no activation-table reloads ever happen.
  - inputs land via few large pair-packed DMAs spread over three DGE
    queues (SP/Pool/ACT) - descriptor generation is ~1.2us serialized per
    DMA per queue, so queue parallelism sets the startup latency.
  - dummy matmuls at t=0 ramp the PE p-state during the input DMA wait.
  - S = K^T blocks x Q (contraction Dh=64) in bf16, exp on ACT, causal
    triangle masked after exp via gpsimd affine_select; PV in natural
    orientation with a ones-column for the softmax denominator; the
    normalize is one reciprocal + one broadcast multiply.
  - phase-interleaved emission: qkv row-blocks and the output projection
    are spread between attention units so the PE stays fed while ACT
    grinds through exp (ACT exp is ~73us of the ~128us critical path);
    extras lead into each phase early (lead=0.45) so their DVE/transpose
    chains land before the next phase's S units need them, and phase-end
    PV chunks use the idle qp psum pool to dodge the sp-bank exp drain.
Host: centers+scales+splits the weights, pair-packs fp8 operands, sums
the 4 partial projections per batch, adds b_proj.
"""

import numpy as np
from contextlib import ExitStack

import ml_dtypes
import concourse.bass as bass
import concourse.mybir as mybir
import concourse.tile as tile
from concourse import bacc
from concourse.bass_utils import run_bass_kernel_spmd

B, N, C, H, Dh = 2, 2048, 1024, 16, 64
HPC = 4                      # heads per core
NCORES = 8
SCALE = 8.0 / Dh             # 0.125 (use_mup)
EPS = 1e-5
WS = 32.0                    # host weight scale for fp8
EPS2 = EPS * WS * WS

F32 = mybir.dt.float32
BF16 = mybir.dt.bfloat16
E4 = mybir.dt.float8e4
AF = mybir.ActivationFunctionType
OP = mybir.AluOpType
DR = mybir.MatmulPerfMode.DoubleRow
BF = ml_dtypes.bfloat16
E4NP = ml_dtypes.float8_e4m3

NB = N // 128                # 16 row blocks of 128
CB = C // 128                # 8 contraction blocks
IB = N // 512                # 4 query blocks of 512
WQ = 768                     # 256 q | 256 k | 256 v

_BUILD_CACHE = {}
LAST_RESULT = None


def _bc3(ap2d, inner):
    """[p, g] AP -> [p, g, inner] with stride-0 inner dim."""
    return bass.AP(tensor=ap2d.tensor, offset=ap2d.offset,
                   ap=list(ap2d.ap) + [[0, inner]])


def _build(causal: bool, fast_gb: bool, exp_bias: float):
    nc = bacc.Bacc("TRN2", target_bir_lowering=False, debug=False,
                   num_devices=NCORES)

    xh_e = nc.dram_tensor("xh8", [512, 2 * N], E4, kind="ExternalInput")
    xl_e = nc.dram_tensor("xl8", [512, 2 * N], E4, kind="ExternalInput")
    wh_e = nc.dram_tensor("wh8", [512, 2 * WQ], E4, kind="ExternalInput")
    wl_e = nc.dram_tensor("wl8", [512, 2 * WQ], E4, kind="ExternalInput")
    wp_e = nc.dram_tensor("wp_t", [HPC * Dh, C], BF16, kind="ExternalInput")
    if not fast_gb:
        gt_e = nc.dram_tensor("g_bcast", [128, 512], F32, kind="ExternalInput")
        bt_e = nc.dram_tensor("b_bcast", [128, 512], F32, kind="ExternalInput")
    out_e = nc.dram_tensor("out_p", [N, C], BF16, kind="ExternalOutput")

    with tile.TileContext(nc) as tc, ExitStack() as ctx:
        persist = ctx.enter_context(tc.tile_pool(name="persist", bufs=1))
        ones_t = persist.tile([128, 1], BF16, tag="ones")
        nc.vector.memset(ones_t[:], 1.0)
        eps_t = persist.tile([128, 1], F32, tag="eps")
        nc.vector.memset(eps_t[:], EPS2)
        eb_t = persist.tile([128, 1], F32, tag="ebias")
        nc.vector.memset(eb_t[:], exp_bias)
        dummy_t = persist.tile([128, 512], BF16, tag="dummy")
        nc.gpsimd.memset(dummy_t[:], 0.125)

        # preload the one ACT table that holds Exp+Ln+Copy+Square (id 6 =
        # natural_log_exp_and_others in act_info.json) so the table-load
        # insertion pass never has to switch tables mid-kernel
        nc.scalar.add_instruction(mybir.InstLoadActFuncSet(
            name=nc.get_next_instruction_name(), act_func_set_id=6,
            engine=mybir.EngineType.Activation, ins=[], outs=[]))

        # transposed q|k, segments: 0,1 = q head-pairs; 2,3 = k head-pairs
        qkT = persist.tile([128, 4, N], BF16, tag="qkT")
        # transposed normalized attention output, head-pairs, input to proj
        oT = persist.tile([128, 2, N], BF16, tag="oT")

        # pair-packed fp8 inputs: [128, pair, k-subtile, cols]
        xha = persist.tile([128, 4, 2, N], E4, tag="xha")
        xla = persist.tile([128, 4, 2, N], E4, tag="xla")
        wha = persist.tile([128, 4, 2, WQ], E4, tag="wha")
        wla = persist.tile([128, 4, 2, WQ], E4, tag="wla")
        wp_t = [persist.tile([128, C], BF16, tag=f"wp{p}", name=f"wp{p}")
                for p in range(2)]

        if not fast_gb:
            gt = persist.tile([128, 512], F32, tag="gt")
            bt = persist.tile([128, 512], F32, tag="bt")
            nc.sync.dma_start(gt[:], gt_e[:])
            nc.sync.dma_start(bt[:], bt_e[:])

        va_pool = ctx.enter_context(tc.tile_pool(name="va", bufs=NB))
        va = [None] * NB

        ptp = ctx.enter_context(tc.tile_pool(name="pt", bufs=44))
        natp = ctx.enter_context(tc.tile_pool(name="nat", bufs=5))
        sqp = ctx.enter_context(tc.tile_pool(name="sq", bufs=6))
        stp = ctx.enter_context(tc.tile_pool(name="st", bufs=32))
        osp = ctx.enter_context(tc.tile_pool(name="os", bufs=6))
        obp = ctx.enter_context(tc.tile_pool(name="ob", bufs=8))
        # PSUM: qp serves qkv blocks + proj halves; sp serves S tiles + o
        qp = ctx.enter_context(tc.tile_pool(name="qp", bufs=2, space="PSUM"))
        sp = ctx.enter_context(tc.tile_pool(name="sp", bufs=2, space="PSUM"))

        # ---- PE warmup: dummies ramp the p-state during the DMA wait ----
        dps = qp.tile([128, WQ], F32, tag="qkv", name="warm")
        for _ in range(8):
            nc.tensor.matmul(dps[:, 0:512], dummy_t[:, 0:128], dummy_t[:],
                             start=True, stop=True)

        # ---- input DMAs across three DGE queues (SP / Pool / ACT) ----
        def drs(e):
            return e[:].rearrange("(p r) (s n) -> r p s n", p=4, s=2)

        nc.sync.dma_start(wha[:], drs(wh_e))
        nc.gpsimd.dma_start(wla[:], drs(wl_e))
        for p in range(2):
            nc.sync.dma_start(xha[:, p, :, 0:512],
                              drs(xh_e)[:, p, :, 0:512])
            nc.gpsimd.dma_start(xla[:, p, :, 0:512],
                                drs(xl_e)[:, p, :, 0:512])
        for p in range(2, 4):
            nc.scalar.dma_start(xha[:, p, :, 0:512],
                                drs(xh_e)[:, p, :, 0:512])
            nc.scalar.dma_start(xla[:, p, :, 0:512],
                                drs(xl_e)[:, p, :, 0:512])
        # second wave split: cols 512:1024 are needed by qkv nb4-7 early
        # in phase 0, so they go first (and partly on the ACT queue)
        for p in range(2):
            nc.sync.dma_start(xha[:, p, :, 512:1024],
                              drs(xh_e)[:, p, :, 512:1024])
            nc.gpsimd.dma_start(xla[:, p, :, 512:1024],
                                drs(xl_e)[:, p, :, 512:1024])
        for p in range(2, 4):
            nc.scalar.dma_start(xha[:, p, :, 512:1024],
                                drs(xh_e)[:, p, :, 512:1024])
            nc.scalar.dma_start(xla[:, p, :, 512:1024],
                                drs(xl_e)[:, p, :, 512:1024])
        for p in range(4):
            nc.sync.dma_start(xha[:, p, :, 1024:N],
                              drs(xh_e)[:, p, :, 1024:N])
            nc.gpsimd.dma_start(xla[:, p, :, 1024:N],
                                drs(xl_e)[:, p, :, 1024:N])
        for p in range(2):
            nc.scalar.dma_start(wp_t[p][:], wp_e[128 * p:128 * (p + 1), :])

        # ---- qkv (3-term fp8 DoubleRow) + LN + transposes + v_aug ----
        TERMS = ((xha, wha), (xha, wla), (xla, wha))   # hh, hl, lh

        def emit_nb(nb):
            n0 = 128 * nb
            qps = qp.tile([128, WQ], F32, tag="qkv", name=f"qkv{nb}")
            for term, (xa, wa) in enumerate(TERMS):
                for p in range(4):
                    st = (term == 0 and p == 0)
                    spf = (term == 2 and p == 3)
                    nc.tensor.matmul(qps[:, 0:512],
                                     xa[:, p, :, n0:n0 + 128],
                                     wa[:, p, :, 0:512], start=st, stop=spf,
                                     perf_mode=DR)
                    nc.tensor.matmul(qps[:, 512:WQ],
                                     xa[:, p, :, n0:n0 + 128],
                                     wa[:, p, :, 512:WQ], start=st,
                                     stop=spf, perf_mode=DR)
            nat = natp.tile([128, WQ], BF16, tag="nat", name=f"nat{nb}")
            sq = sqp.tile([128, 512], BF16, tag="sq", name=f"sq{nb}")
            if nb < 4:
                # prologue: ACT is idle until the first exp
                nc.scalar.activation(nat[:], qps[:], func=AF.Copy)
                nc.scalar.activation(sq[:], nat[:, 0:512], func=AF.Square)
            else:
                nc.vector.tensor_copy(nat[:], qps[:])
                nc.vector.tensor_tensor(sq[:], nat[:, 0:512], nat[:, 0:512],
                                        op=OP.mult)

            # centered weights: var = mean(q_c^2); rstd folds the /64 into
            # the Ln scale, eps' absorbs the x32 weight scaling
            rstd = stp.tile([128, 8], F32, tag="rstd", name=f"rstd{nb}")
            nc.vector.tensor_reduce(rstd[:],
                                    sq[:].rearrange("p (g d) -> p g d", g=8),
                                    axis=mybir.AxisListType.X, op=OP.add)
            nc.scalar.activation(rstd[:], rstd[:], func=AF.Ln,
                                 scale=1.0 / Dh, bias=eps_t[:])
            nc.scalar.activation(rstd[:], rstd[:], func=AF.Exp, scale=-0.5)

            qk3 = nat[:, 0:512].rearrange("p (g d) -> p g d", g=8)
            nc.vector.tensor_tensor(qk3, qk3, _bc3(rstd[:], Dh), op=OP.mult)
            if not fast_gb:
                nc.vector.tensor_tensor(nat[:, 0:512], nat[:, 0:512], gt[:],
                                        op=OP.mult)
                nc.vector.tensor_tensor(nat[:, 0:512], nat[:, 0:512], bt[:],
                                        op=OP.add)

            nc.sync.dma_start(qkT[:, :, n0:n0 + 128], nat[:, 0:512],
                              transpose=True)

            vat = va_pool.tile([128, HPC, Dh + 1], BF16, tag="vat",
                               name=f"vat{nb}")
            nc.gpsimd.tensor_copy(
                vat[:, :, 0:Dh],
                nat[:, 512:768].rearrange("p (h d) -> p h d", h=HPC))
            nc.gpsimd.tensor_copy(vat[:, :, Dh:Dh + 1],
                                  ones_t[:].to_broadcast([128, HPC, 1]))
            va[nb] = vat

        def width(ib, jb):
            if not causal or jb < 4 * ib:
                return 512
            return 512 - 128 * (jb - 4 * ib)

        # ---- S + exp + mask for one (ib, h, jp); returns the pt tile ----
        def emit_sjp(ib, h, jp):
            p, off = h // 2, 64 * (h % 2)
            i0 = 512 * ib
            jbs = (2 * jp, 2 * jp + 1)
            ws = [width(ib, jb) for jb in jbs]
            # storage: half0 right-aligned to 512, half1 left-aligned at 512
            # -> live cols [512-ws0, 512+ws1) always contiguous
            s_ps = sp.tile([128, 1024], F32, tag="sps",
                           name=f"s{ib}_{h}_{jp}")
            c0s = [512 - ws[0], 512]
            for half, jb in enumerate(jbs):
                w = ws[half]
                nc.tensor.matmul(
                    s_ps[:, c0s[half]:c0s[half] + w],
                    qkT[off:off + Dh, 2 + p, 128 * jb:128 * (jb + 1)],
                    qkT[off:off + Dh, p, i0 + 512 - w:i0 + 512],
                    start=True, stop=True)
            pt = ptp.tile([128, 1024], BF16, tag="pt",
                          name=f"pt{ib}_{h}_{jp}")
            ebias = 0.0 if exp_bias == 0.0 else eb_t[:]
            nc.scalar.activation(pt[:, c0s[0]:512 + ws[1]],
                                 s_ps[:, c0s[0]:512 + ws[1]],
                                 func=AF.Exp, scale=SCALE, bias=ebias)
            for half, jb in enumerate(jbs):
                if causal and jb >= 4 * ib:
                    # triangle lives in the first live 128 cols of this half
                    t = 128 * (jb - 4 * ib)
                    c = (t if half == 0 else 512)
                    nc.gpsimd.affine_select(
                        out=pt[:, c:c + 128], in_=pt[:, c:c + 128],
                        compare_op=OP.is_ge, fill=0.0, base=0,
                        pattern=[[1, 128]], channel_multiplier=-1)
            return pt

        # pt column of block jb for query chunk g (global 128-chunk index)
        def pt_col(ib, jb, half, g):
            bp = g - 4 * ib
            if half == 0:
                return 128 * bp          # right-aligned or full: col = 128*bp
            return 512 + 128 * bp - (512 - width(ib, jb))

        # ---- PV + normalize + O-transpose for one i-chunk g ----
        def emit_pv_chunk(ib, g, pts, psum_pool):
            jmax = g + 1 if causal else NB
            o_t = psum_pool.tile(
                [128, 1024] if psum_pool is sp else [128, WQ], F32,
                tag=("sps" if psum_pool is sp else "qkv"), name=f"o{g}")
            o_ps = o_t[:, 0:HPC * (Dh + 1)].rearrange(
                "p (h d) -> p h d", h=HPC)
            for h in range(HPC):
                for jb in range(jmax):
                    jp, half = jb // 2, jb % 2
                    col = pt_col(ib, jb, half, g)
                    pt = pts[(h, jp)]
                    nc.tensor.matmul(
                        o_ps[:, h, :], pt[:, col:col + 128],
                        va[jb][:, h, :],
                        start=(jb == 0), stop=(jb == jmax - 1))
            rd = stp.tile([128, HPC, 1], F32, tag="rd", name=f"rd{g}")
            nc.vector.reciprocal(rd[:], o_ps[:, :, Dh:Dh + 1])
            osb = osp.tile([128, HPC, Dh], BF16, tag="osb", name=f"osb{g}")
            nc.vector.tensor_tensor(osb[:], o_ps[:, :, 0:Dh],
                                    _bc3(rd[:, :, 0], Dh), op=OP.mult)
            n0 = 128 * g
            nc.sync.dma_start(oT[:, :, n0:n0 + 128], osb[:],
                              transpose=True)

        # ---- output projection, one 512-col half at a time ----
        proj_ob = {}

        def emit_proj_half(nb, j2):
            n0 = 128 * nb
            if nb not in proj_ob:
                proj_ob[nb] = obp.tile([128, C], BF16, tag="ob",
                                       name=f"ob{nb}")
            ob = proj_ob[nb]
            if True:
                pp_t = qp.tile([128, WQ], F32, tag="qkv", name=f"pp{nb}_{j2}")
                pp = pp_t[:, 0:512]
                nc.tensor.matmul(pp[:], oT[:, 0, n0:n0 + 128],
                                 wp_t[0][:, 512 * j2:512 * (j2 + 1)],
                                 start=True, stop=False)
                nc.tensor.matmul(pp[:], oT[:, 1, n0:n0 + 128],
                                 wp_t[1][:, 512 * j2:512 * (j2 + 1)],
                                 start=False, stop=True)
                if nb >= 12:
                    # tail: ACT is idle once the last exps drain
                    nc.scalar.activation(ob[:, 512 * j2:512 * (j2 + 1)],
                                         pp[:], func=AF.Copy)
                else:
                    nc.vector.tensor_copy(ob[:, 512 * j2:512 * (j2 + 1)],
                                          pp[:])
            if j2 == 1:
                nc.sync.dma_start(out_e[n0:n0 + 128, :], ob[:])
                proj_ob.pop(nb)

        def emit_proj(nb):
            emit_proj_half(nb, 0)
            emit_proj_half(nb, 1)

        # ---- interleaved emission ----
        def interleave(units, extras, lead=2.0):
            if not extras:
                for u in units:
                    u()
                return
            k = len(units) / (len(extras) + 1)
            nxt, ei = k * lead, 0
            for i, u in enumerate(units):
                u()
                while ei < len(extras) and i + 1 >= nxt:
                    extras[ei]()
                    ei += 1
                    nxt += k
            while ei < len(extras):
                extras[ei]()
                ei += 1

        for nb in range(4):
            emit_nb(nb)
        proj_sched = {3: [0, 1, 2]}
        for k in range(IB):
            pts = {}
            units = []
            jmax = 4 * (k + 1) if causal else NB
            for h in range(HPC):
                for jp in range(jmax // 2):
                    units.append(lambda ib=k, h=h, jp=jp:
                                 pts.__setitem__((h, jp), emit_sjp(ib, h, jp)))
            extras = []
            if k < IB - 1:
                extras += [lambda nb=nb: emit_nb(nb)
                           for nb in range(4 * (k + 1), 4 * (k + 2))]
            for pib in proj_sched.get(k, []):
                extras += [lambda nb=nb: emit_proj(nb)
                           for nb in range(4 * pib, 4 * pib + 4)]
            interleave(units, extras, lead=0.45)
            # first two PV chunks use qp psum (free at phase end) so they
            # don't wait for the trailing exps to release the sp banks
            for g in range(4 * k, 4 * k + 4):
                emit_pv_chunk(k, g, pts, qp if g < 4 * k + 2 else sp)
        for nb in range(4 * (IB - 1), 4 * IB):
            emit_proj(nb)
    return nc


def kernel(x, W_qkv, W_proj, b_proj, ln_g, ln_b, causal, _trace=False):
    global LAST_RESULT
    x = np.asarray(x, dtype=np.float32)
    W_qkv = np.asarray(W_qkv, dtype=np.float32)
    W_proj = np.asarray(W_proj, dtype=np.float32)
    b_proj = np.asarray(b_proj, dtype=np.float32)
    ln_g = np.asarray(ln_g, dtype=np.float32)
    ln_b = np.asarray(ln_b, dtype=np.float32)
    causal = bool(int(np.asarray(causal)))

    fast_gb = bool(np.all(ln_g == 1.0) and np.all(ln_b == 0.0))
    exp_bias = 0.0
    if not fast_gb:
        m = float(SCALE * (8.0 * np.abs(ln_g).max() + 8.0 * np.abs(ln_b).max()) ** 2)
        exp_bias = -max(0.0, m - 8.0)

    key = (causal, fast_gb, exp_bias)
    if key not in _BUILD_CACHE:
        nc = _build(causal, fast_gb, exp_bias)
        nc.finalize()
        _BUILD_CACHE[key] = nc
    nc = _BUILD_CACHE[key]

    def pairpack(a):
        # [1024, M] -> rows (256p + 128s + r) -> [512, 2M] with r-major rows
        M = a.shape[1]
        return np.ascontiguousarray(
            a.reshape(4, 2, 128, M).transpose(0, 2, 1, 3).reshape(512, 2 * M))

    def split8(a):
        hi = a.astype(E4NP)
        lo = (a - hi.astype(np.float32)).astype(E4NP)
        return hi, lo

    # center q,k weight rows per head; scale by WS so fp8 splits stay in
    # normal range (LN self-corrects via eps'; v-path folds into wp/WS)
    Wc = W_qkv.copy()
    for part in range(2):
        blk = Wc[part * C:(part + 1) * C].reshape(H, Dh, C)
        Wc[part * C:(part + 1) * C] = (
            blk - blk.mean(axis=1, keepdims=True)).reshape(C, C)
    Wsc = Wc * WS

    xts = []
    for b in range(B):
        xt = np.ascontiguousarray(x[b].T)            # [C, N]
        hi, lo = split8(xt)
        xts.append((pairpack(hi.view(np.uint8)).view(E4NP),
                    pairpack(lo.view(np.uint8)).view(E4NP)))

    in_maps = []
    for c in range(NCORES):
        b, h0 = c // HPC, Dh * HPC * (c % HPC)   # h0 in channel units
        rq = Wsc[h0:h0 + 256]
        rk = Wsc[C + h0:C + h0 + 256]
        rv = Wsc[2 * C + h0:2 * C + h0 + 256]
        w_all = np.concatenate([rq, rk, rv])          # [768, 1024]
        wT = np.ascontiguousarray(w_all.T)            # [1024, 768]
        whv, wlv = split8(wT)
        im = {
            "xh8": xts[b][0],
            "xl8": xts[b][1],
            "wh8": pairpack(whv.view(np.uint8)).view(E4NP),
            "wl8": pairpack(wlv.view(np.uint8)).view(E4NP),
            "wp_t": np.ascontiguousarray(
                (W_proj[:, h0:h0 + 256] / WS).T).astype(BF),
        }
        if not fast_gb:
            gseg = np.tile(ln_g, 8)              # q heads x4 then k heads x4
            bseg = np.tile(ln_b, 8)
            im["g_bcast"] = np.broadcast_to(gseg, (128, 512)).copy()
            im["b_bcast"] = np.broadcast_to(bseg, (128, 512)).copy()
        in_maps.append(im)

    res = run_bass_kernel_spmd(nc, in_maps, core_ids=list(range(NCORES)),
                               trace=_trace)
    LAST_RESULT = res

    out = np.empty((B, N, C), dtype=np.float32)
    for b in range(B):
        acc = res.results[4 * b]["out_p"].astype(np.float32)
        for c in range(4 * b + 1, 4 * b + 4):
            acc = acc + res.results[c]["out_p"].astype(np.float32)
        out[b] = acc + b_proj
    return out



# revision 89
# speedup vs baseline: 1.0013x; 1.0013x over previous
"""Trainium2 Bass kernel for BasicSelfAttention (B=2, N=2048, C=1024, H=16, Dh=64).

Sharding: 8 cores = 2 batches x 4 head-groups. Core c handles batch c//4 and
heads [4*(c%4), 4*(c%4)+4).

v3 design (on top of the v2 all-bf16 phase-interleaved kernel):
  - qkv projection in fp8-e4m3 DoubleRow mode (0.5 PE cycles/row, 2x bf16)
    with a 3-term residual split: x ~ xh+xl, W ~ wh+wl quantized at the
    SAME scale so all terms accumulate raw in one PSUM group;
    qkv = xh@wh + xh@wl + xl@wh. The dropped xl@wl term is ~2^-8 and
    incoherent, so accuracy stays at bf16 level (rel err 3.7e-3).
  - q/k weight rows are centered host-side (head-mean removed), which
    deletes the whole device-side LN mean path: var = mean(q_c^2), and
    the LN application is a single broadcast multiply.
  - W scaled x32 so the fp8 residual splits stay in e4m3 normal range;
    LN is scale-invariant (eps' = eps*32^2 folded into the Ln bias), the
    v-path scale folds into W_proj/32 host-side.
  - rstd = exp(-0.5*ln(var/64+eps')): Ln/Exp/Copy/Square share one ACT
    table (id 6), so no activation-table reloads ever happen.
  - inputs land via few large pair-packed DMAs spread over three DGE
    queues (SP/Pool/ACT) - descriptor generation is ~1.2us serialized per
    DMA per queue, so queue parallelism sets the startup latency.
  - dummy matmuls at t=0 ramp the PE p-state during the input DMA wait.
  - S = K^T blocks x Q (contraction Dh=64) in bf16, exp on ACT, causal
    triangle masked after exp via gpsimd affine_select; PV in natural
    orientation with a ones-column for the softmax denominator; the
    normalize is one reciprocal + one broadcast multiply.
  - phase-interleaved emission: qkv row-blocks and the output projection
    are spread between attention units so the PE stays fed while ACT
    grinds through exp (ACT exp is ~73us of the ~128us critical path);
    extras lead into each phase early (lead=0.45) so their DVE/transpose
    chains land before the next phase's S units need them, and phase-end
    PV chunks use the idle qp psum pool to dodge the sp-bank exp drain.
Host: centers+scales+splits the weights, pair-packs fp8 operands, sums
the 4 partial projections per batch, adds b_proj.
"""

import numpy as np
from contextlib import ExitStack

import ml_dtypes
import concourse.bass as bass
import concourse.mybir as mybir
import concourse.tile as tile
from concourse import bacc
from concourse.bass_utils import run_bass_kernel_spmd

B, N, C, H, Dh = 2, 2048, 1024, 16, 64
HPC = 4                      # heads per core
NCORES = 8
SCALE = 8.0 / Dh             # 0.125 (use_mup)
EPS = 1e-5
WS = 32.0                    # host weight scale for fp8
EPS2 = EPS * WS * WS

F32 = mybir.dt.float32
BF16 = mybir.dt.bfloat16
E4 = mybir.dt.float8e4
AF = mybir.ActivationFunctionType
OP = mybir.AluOpType
DR = mybir.MatmulPerfMode.DoubleRow
BF = ml_dtypes.bfloat16
E4NP = ml_dtypes.float8_e4m3

NB = N // 128                # 16 row blocks of 128
CB = C // 128                # 8 contraction blocks
IB = N // 512                # 4 query blocks of 512
WQ = 768                     # 256 q | 256 k | 256 v

_BUILD_CACHE = {}
LAST_RESULT = None


def _bc3(ap2d, inner):
    """[p, g] AP -> [p, g, inner] with stride-0 inner dim."""
    return bass.AP(tensor=ap2d.tensor, offset=ap2d.offset,
                   ap=list(ap2d.ap) + [[0, inner]])


def _build(causal: bool, fast_gb: bool, exp_bias: float):
    nc = bacc.Bacc("TRN2", target_bir_lowering=False, debug=False,
                   num_devices=NCORES)

    xh_e = nc.dram_tensor("xh8", [512, 2 * N], E4, kind="ExternalInput")
    xl_e = nc.dram_tensor("xl8", [512, 2 * N], E4, kind="ExternalInput")
    wh_e = nc.dram_tensor("wh8", [512, 2 * WQ], E4, kind="ExternalInput")
    wl_e = nc.dram_tensor("wl8", [512, 2 * WQ], E4, kind="ExternalInput")
    wp_e = nc.dram_tensor("wp_t", [HPC * Dh, C], BF16, kind="ExternalInput")
    if not fast_gb:
        gt_e = nc.dram_tensor("g_bcast", [128, 512], F32, kind="ExternalInput")
        bt_e = nc.dram_tensor("b_bcast", [128, 512], F32, kind="ExternalInput")
    out_e = nc.dram_tensor("out_p", [N, C], BF16, kind="ExternalOutput")

    with tile.TileContext(nc) as tc, ExitStack() as ctx:
        persist = ctx.enter_context(tc.tile_pool(name="persist", bufs=1))
        ones_t = persist.tile([128, 1], BF16, tag="ones")
        nc.vector.memset(ones_t[:], 1.0)
        eps_t = persist.tile([128, 1], F32, tag="eps")
        nc.vector.memset(eps_t[:], EPS2)
        eb_t = persist.tile([128, 1], F32, tag="ebias")
        nc.vector.memset(eb_t[:], exp_bias)
        dummy_t = persist.tile([128, 512], BF16, tag="dummy")
        nc.gpsimd.memset(dummy_t[:], 0.125)

        # preload the one ACT table that holds Exp+Ln+Copy+Square (id 6 =
        # natural_log_exp_and_others in act_info.json) so the table-load
        # insertion pass never has to switch tables mid-kernel
        nc.scalar.add_instruction(mybir.InstLoadActFuncSet(
            name=nc.get_next_instruction_name(), act_func_set_id=6,
            engine=mybir.EngineType.Activation, ins=[], outs=[]))

        # transposed q|k, segments: 0,1 = q head-pairs; 2,3 = k head-pairs
        qkT = persist.tile([128, 4, N], BF16, tag="qkT")
        # transposed normalized attention output, head-pairs, input to proj
        oT = persist.tile([128, 2, N], BF16, tag="oT")

        # pair-packed fp8 inputs: [128, pair, k-subtile, cols]
        xha = persist.tile([128, 4, 2, N], E4, tag="xha")
        xla = persist.tile([128, 4, 2, N], E4, tag="xla")
        wha = persist.tile([128, 4, 2, WQ], E4, tag="wha")
        wla = persist.tile([128, 4, 2, WQ], E4, tag="wla")
        wp_t = [persist.tile([128, C], BF16, tag=f"wp{p}", name=f"wp{p}")
                for p in range(2)]

        if not fast_gb:
            gt = persist.tile([128, 512], F32, tag="gt")
            bt = persist.tile([128, 512], F32, tag="bt")
            nc.sync.dma_start(gt[:], gt_e[:])
            nc.sync.dma_start(bt[:], bt_e[:])

        va_pool = ctx.enter_context(tc.tile_pool(name="va", bufs=NB))
        va = [None] * NB

        ptp = ctx.enter_context(tc.tile_pool(name="pt", bufs=40))
        natp = ctx.enter_context(tc.tile_pool(name="nat", bufs=6))
        sqp = ctx.enter_context(tc.tile_pool(name="sq", bufs=6))
        stp = ctx.enter_context(tc.tile_pool(name="st", bufs=32))
        osp = ctx.enter_context(tc.tile_pool(name="os", bufs=6))
        obp = ctx.enter_context(tc.tile_pool(name="ob", bufs=10))
        # PSUM: qp serves qkv blocks + proj halves; sp serves S tiles + o
        qp = ctx.enter_context(tc.tile_pool(name="qp", bufs=2, space="PSUM"))
        sp = ctx.enter_context(tc.tile_pool(name="sp", bufs=2, space="PSUM"))

        # ---- PE warmup: dummies ramp the p-state during the DMA wait ----
        dps = qp.tile([128, WQ], F32, tag="qkv", name="warm")
        for _ in range(8):
            nc.tensor.matmul(dps[:, 0:512], dummy_t[:, 0:128], dummy_t[:],
                             start=True, stop=True)

        # ---- input DMAs across three DGE queues (SP / Pool / ACT) ----
        def drs(e):
            return e[:].rearrange("(p r) (s n) -> r p s n", p=4, s=2)

        nc.sync.dma_start(wha[:], drs(wh_e))
        nc.gpsimd.dma_start(wla[:], drs(wl_e))
        for p in range(2):
            nc.sync.dma_start(xha[:, p, :, 0:512],
                              drs(xh_e)[:, p, :, 0:512])
            nc.gpsimd.dma_start(xla[:, p, :, 0:512],
                                drs(xl_e)[:, p, :, 0:512])
        for p in range(2, 4):
            nc.scalar.dma_start(xha[:, p, :, 0:512],
                                drs(xh_e)[:, p, :, 0:512])
            nc.scalar.dma_start(xla[:, p, :, 0:512],
                                drs(xl_e)[:, p, :, 0:512])
        # second wave split: cols 512:1024 are needed by qkv nb4-7 early
        # in phase 0, so they go first (and partly on the ACT queue)
        for p in range(2):
            nc.sync.dma_start(xha[:, p, :, 512:1024],
                              drs(xh_e)[:, p, :, 512:1024])
            nc.gpsimd.dma_start(xla[:, p, :, 512:1024],
                                drs(xl_e)[:, p, :, 512:1024])
        for p in range(2, 4):
            nc.scalar.dma_start(xha[:, p, :, 512:1024],
                                drs(xh_e)[:, p, :, 512:1024])
            nc.scalar.dma_start(xla[:, p, :, 512:1024],
                                drs(xl_e)[:, p, :, 512:1024])
        for p in range(4):
            nc.sync.dma_start(xha[:, p, :, 1024:N],
                              drs(xh_e)[:, p, :, 1024:N])
            nc.gpsimd.dma_start(xla[:, p, :, 1024:N],
                                drs(xl_e)[:, p, :, 1024:N])
        for p in range(2):
            nc.scalar.dma_start(wp_t[p][:], wp_e[128 * p:128 * (p + 1), :])

        # ---- qkv (3-term fp8 DoubleRow) + LN + transposes + v_aug ----
        TERMS = ((xha, wha), (xha, wla), (xla, wha))   # hh, hl, lh

        def emit_nb(nb):
            n0 = 128 * nb
            qps = qp.tile([128, WQ], F32, tag="qkv", name=f"qkv{nb}")
            for term, (xa, wa) in enumerate(TERMS):
                for p in range(4):
                    st = (term == 0 and p == 0)
                    spf = (term == 2 and p == 3)
                    nc.tensor.matmul(qps[:, 0:512],
                                     xa[:, p, :, n0:n0 + 128],
                                     wa[:, p, :, 0:512], start=st, stop=spf,
                                     perf_mode=DR)
                    nc.tensor.matmul(qps[:, 512:WQ],
                                     xa[:, p, :, n0:n0 + 128],
                                     wa[:, p, :, 512:WQ], start=st,
                                     stop=spf, perf_mode=DR)
            nat = natp.tile([128, WQ], BF16, tag="nat", name=f"nat{nb}")
            sq = sqp.tile([128, 512], BF16, tag="sq", name=f"sq{nb}")
            if nb < 4:
                # prologue: ACT is idle until the first exp
                nc.scalar.activation(nat[:], qps[:], func=AF.Copy)
                nc.scalar.activation(sq[:], nat[:, 0:512], func=AF.Square)
            else:
                nc.vector.tensor_copy(nat[:], qps[:])
                nc.vector.tensor_tensor(sq[:], nat[:, 0:512], nat[:, 0:512],
                                        op=OP.mult)

            # centered weights: var = mean(q_c^2); rstd folds the /64 into
            # the Ln scale, eps' absorbs the x32 weight scaling
            rstd = stp.tile([128, 8], F32, tag="rstd", name=f"rstd{nb}")
            nc.vector.tensor_reduce(rstd[:],
                                    sq[:].rearrange("p (g d) -> p g d", g=8),
                                    axis=mybir.AxisListType.X, op=OP.add)
            nc.scalar.activation(rstd[:], rstd[:], func=AF.Ln,
                                 scale=1.0 / Dh, bias=eps_t[:])
            nc.scalar.activation(rstd[:], rstd[:], func=AF.Exp, scale=-0.5)

            qk3 = nat[:, 0:512].rearrange("p (g d) -> p g d", g=8)
            nc.vector.tensor_tensor(qk3, qk3, _bc3(rstd[:], Dh), op=OP.mult)
            if not fast_gb:
                nc.vector.tensor_tensor(nat[:, 0:512], nat[:, 0:512], gt[:],
                                        op=OP.mult)
                nc.vector.tensor_tensor(nat[:, 0:512], nat[:, 0:512], bt[:],
                                        op=OP.add)

            nc.sync.dma_start(qkT[:, :, n0:n0 + 128], nat[:, 0:512],
                              transpose=True)

            vat = va_pool.tile([128, HPC, Dh + 1], BF16, tag="vat",
                               name=f"vat{nb}")
            nc.gpsimd.tensor_copy(
                vat[:, :, 0:Dh],
                nat[:, 512:768].rearrange("p (h d) -> p h d", h=HPC))
            nc.gpsimd.tensor_copy(vat[:, :, Dh:Dh + 1],
                                  ones_t[:].to_broadcast([128, HPC, 1]))
            va[nb] = vat

        def width(ib, jb):
            if not causal or jb < 4 * ib:
                return 512
            return 512 - 128 * (jb - 4 * ib)

        # ---- S + exp + mask for one (ib, h, jp); returns the pt tile ----
        def emit_sjp(ib, h, jp):
            p, off = h // 2, 64 * (h % 2)
            i0 = 512 * ib
            jbs = (2 * jp, 2 * jp + 1)
            ws = [width(ib, jb) for jb in jbs]
            # storage: half0 right-aligned to 512, half1 left-aligned at 512
            # -> live cols [512-ws0, 512+ws1) always contiguous
            s_ps = sp.tile([128, 1024], F32, tag="sps",
                           name=f"s{ib}_{h}_{jp}")
            c0s = [512 - ws[0], 512]
            for half, jb in enumerate(jbs):
                w = ws[half]
                nc.tensor.matmul(
                    s_ps[:, c0s[half]:c0s[half] + w],
                    qkT[off:off + Dh, 2 + p, 128 * jb:128 * (jb + 1)],
                    qkT[off:off + Dh, p, i0 + 512 - w:i0 + 512],
                    start=True, stop=True)
            pt = ptp.tile([128, 1024], BF16, tag="pt",
                          name=f"pt{ib}_{h}_{jp}")
            ebias = 0.0 if exp_bias == 0.0 else eb_t[:]
            nc.scalar.activation(pt[:, c0s[0]:512 + ws[1]],
                                 s_ps[:, c0s[0]:512 + ws[1]],
                                 func=AF.Exp, scale=SCALE, bias=ebias)
            for half, jb in enumerate(jbs):
                if causal and jb >= 4 * ib:
                    # triangle lives in the first live 128 cols of this half
                    t = 128 * (jb - 4 * ib)
                    c = (t if half == 0 else 512)
                    nc.gpsimd.affine_select(
                        out=pt[:, c:c + 128], in_=pt[:, c:c + 128],
                        compare_op=OP.is_ge, fill=0.0, base=0,
                        pattern=[[1, 128]], channel_multiplier=-1)
            return pt

        # pt column of block jb for query chunk g (global 128-chunk index)
        def pt_col(ib, jb, half, g):
            bp = g - 4 * ib
            if half == 0:
                return 128 * bp          # right-aligned or full: col = 128*bp
            return 512 + 128 * bp - (512 - width(ib, jb))

        # ---- PV + normalize + O-transpose for one i-chunk g ----
        def emit_pv_chunk(ib, g, pts, psum_pool):
            jmax = g + 1 if causal else NB
            o_t = psum_pool.tile(
                [128, 1024] if psum_pool is sp else [128, WQ], F32,
                tag=("sps" if psum_pool is sp else "qkv"), name=f"o{g}")
            o_ps = o_t[:, 0:HPC * (Dh + 1)].rearrange(
                "p (h d) -> p h d", h=HPC)
            for h in range(HPC):
                for jb in range(jmax):
                    jp, half = jb // 2, jb % 2
                    col = pt_col(ib, jb, half, g)
                    pt = pts[(h, jp)]
                    nc.tensor.matmul(
                        o_ps[:, h, :], pt[:, col:col + 128],
                        va[jb][:, h, :],
                        start=(jb == 0), stop=(jb == jmax - 1))
            rd = stp.tile([128, HPC, 1], F32, tag="rd", name=f"rd{g}")
            nc.vector.reciprocal(rd[:], o_ps[:, :, Dh:Dh + 1])
            osb = osp.tile([128, HPC, Dh], BF16, tag="osb", name=f"osb{g}")
            nc.vector.tensor_tensor(osb[:], o_ps[:, :, 0:Dh],
                                    _bc3(rd[:, :, 0], Dh), op=OP.mult)
            n0 = 128 * g
            nc.sync.dma_start(oT[:, :, n0:n0 + 128], osb[:],
                              transpose=True)

        # ---- output projection, one 512-col half at a time ----
        proj_ob = {}

        def emit_proj_half(nb, j2):
            n0 = 128 * nb
            if nb not in proj_ob:
                proj_ob[nb] = obp.tile([128, C], BF16, tag="ob",
                                       name=f"ob{nb}")
            ob = proj_ob[nb]
            if True:
                pp_t = qp.tile([128, WQ], F32, tag="qkv", name=f"pp{nb}_{j2}")
                pp = pp_t[:, 0:512]
                nc.tensor.matmul(pp[:], oT[:, 0, n0:n0 + 128],
                                 wp_t[0][:, 512 * j2:512 * (j2 + 1)],
                                 start=True, stop=False)
                nc.tensor.matmul(pp[:], oT[:, 1, n0:n0 + 128],
                                 wp_t[1][:, 512 * j2:512 * (j2 + 1)],
                                 start=False, stop=True)
                if nb >= 12:
                    # tail: ACT is idle once the last exps drain
                    nc.scalar.activation(ob[:, 512 * j2:512 * (j2 + 1)],
                                         pp[:], func=AF.Copy)
                else:
                    nc.vector.tensor_copy(ob[:, 512 * j2:512 * (j2 + 1)],
                                          pp[:])
            if j2 == 1:
                nc.sync.dma_start(out_e[n0:n0 + 128, :], ob[:])
                proj_ob.pop(nb)

        def emit_proj(nb):
            emit_proj_half(nb, 0)
            emit_proj_half(nb, 1)

        # ---- interleaved emission ----
        def interleave(units, extras, lead=2.0):
            if not extras:
                for u in units:
                    u()
                return
            k = len(units) / (len(extras) + 1)
            nxt, ei = k * lead, 0
            for i, u in enumerate(units):
                u()
                while ei < len(extras) and i + 1 >= nxt:
                    extras[ei]()
                    ei += 1
                    nxt += k
            while ei < len(extras):
                extras[ei]()
                ei += 1

        for nb in range(4):
            emit_nb(nb)
        proj_sched = {3: [0, 1, 2]}
        for k in range(IB):
            pts = {}
            units = []
            jmax = 4 * (k + 1) if causal else NB
            for h in range(HPC):
                for jp in range(jmax // 2):
                    units.append(lambda ib=k, h=h, jp=jp:
                                 pts.__setitem__((h, jp), emit_sjp(ib, h, jp)))
            extras = []
            if k < IB - 1:
                extras += [lambda nb=nb: emit_nb(nb)
                           for nb in range(4 * (k + 1), 4 * (k + 2))]
            for pib in proj_sched.get(k, []):
                extras += [lambda nb=nb: emit_proj(nb)
                           for nb in range(4 * pib, 4 * pib + 4)]
            interleave(units, extras, lead=0.45)
            # first two PV chunks use qp psum (free at phase end) so they
            # don't wait for the trailing exps to release the sp banks
            for g in range(4 * k, 4 * k + 4):
                emit_pv_chunk(k, g, pts, qp if g < 4 * k + 2 else sp)
        for nb in range(4 * (IB - 1), 4 * IB):
            emit_proj(nb)
    return nc


def kernel(x, W_qkv, W_proj, b_proj, ln_g, ln_b, causal, _trace=False):
    global LAST_RESULT
    x = np.asarray(x, dtype=np.float32)
    W_qkv = np.asarray(W_qkv, dtype=np.float32)
    W_proj = np.asarray(W_proj, dtype=np.float32)
    b_proj = np.asarray(b_proj, dtype=np.float32)
    ln_g = np.asarray(ln_g, dtype=np.float32)
    ln_b = np.asarray(ln_b, dtype=np.float32)
    causal = bool(int(np.asarray(causal)))

    fast_gb = bool(np.all(ln_g == 1.0) and np.all(ln_b == 0.0))
    exp_bias = 0.0
    if not fast_gb:
        m = float(SCALE * (8.0 * np.abs(ln_g).max() + 8.0 * np.abs(ln_b).max()) ** 2)
        exp_bias = -max(0.0, m - 8.0)

    key = (causal, fast_gb, exp_bias)
    if key not in _BUILD_CACHE:
        nc = _build(causal, fast_gb, exp_bias)
        nc.finalize()
        _BUILD_CACHE[key] = nc
    nc = _BUILD_CACHE[key]

    def pairpack(a):
        # [1024, M] -> rows (256p + 128s + r) -> [512, 2M] with r-major rows
        M = a.shape[1]
        return np.ascontiguousarray(
            a.reshape(4, 2, 128, M).transpose(0, 2, 1, 3).reshape(512, 2 * M))

    def split8(a):
        hi = a.astype(E4NP)
        lo = (a - hi.astype(np.float32)).astype(E4NP)
        return hi, lo

    # center q,k weight rows per head; scale by WS so fp8 splits stay in
    # normal range (LN self-corrects via eps'; v-path folds into wp/WS)
    Wc = W_qkv.copy()
    for part in range(2):
        blk = Wc[part * C:(part + 1) * C].reshape(H, Dh, C)
        Wc[part * C:(part + 1) * C] = (
            blk - blk.mean(axis=1, keepdims=True)).reshape(C, C)
    Wsc = Wc * WS

    xts = []
    for b in range(B):
        xt = np.ascontiguousarray(x[b].T)            # [C, N]
        hi, lo = split8(xt)
        xts.append((pairpack(hi.view(np.uint8)).view(E4NP),
                    pairpack(lo.view(np.uint8)).view(E4NP)))

    in_maps = []
    for c in range(NCORES):
        b, h0 = c // HPC, Dh * HPC * (c % HPC)   # h0 in channel units
        rq = Wsc[h0:h0 + 256]
        rk = Wsc[C + h0:C + h0 + 256]
        rv = Wsc[2 * C + h0:2 * C + h0 + 256]
        w_all = np.concatenate([rq, rk, rv])          # [768, 1024]
        wT = np.ascontiguousarray(w_all.T)            # [1024, 768]
        whv, wlv = split8(wT)
        im = {
            "xh8": xts[b][0],
            "xl8": xts[b][1],
            "wh8": pairpack(whv.view(np.uint8)).view(E4NP),
            "wl8": pairpack(wlv.view(np.uint8)).view(E4NP),
            "wp_t": np.ascontiguousarray(
                (W_proj[:, h0:h0 + 256] / WS).T).astype(BF),
        }
        if not fast_gb:
            gseg = np.tile(ln_g, 8)              # q heads x4 then k heads x4
            bseg = np.tile(ln_b, 8)
            im["g_bcast"] = np.broadcast_to(gseg, (128, 512)).copy()
            im["b_bcast"] = np.broadcast_to(bseg, (128, 512)).copy()
        in_maps.append(im)

    res = run_bass_kernel_spmd(nc, in_maps, core_ids=list(range(NCORES)),
                               trace=_trace)
    LAST_RESULT = res

    out = np.empty((B, N, C), dtype=np.float32)
    for b in range(B):
        acc = res.results[4 * b]["out_p"].astype(np.float32)
        for c in range(4 * b + 1, 4 * b + 4):
            acc = acc + res.results[c]["out_p"].astype(np.float32)
        out[b] = acc + b_proj
    return out

